# revision 2
# baseline (speedup 1.0000x reference)
"""KNN classifier layer (B=1024, N=32768, D=64, k=8, C=6) on 8 trn2 cores.

Strategy: shard queries (batch) across the 8 cores, 128 queries per core;
replicate the training set. Per core:
  key[q, n] = x_q . X_n - |X_n|^2/2   (monotone decreasing in distance^2)
computed as one augmented matmul ([x, 1] . [X, -|X|^2/2]), evacuated
PSUM->SBUF by the scalar engine. Top-8 per query = max8 over per-2048-chunk
top-8 candidates (union of chunk top-8s contains the global top-8). The
label histogram needs no indices: X_train is pre-sorted by class on the
host so each class is a contiguous column block; count of keys >= t_q
(t_q = 8th largest key) inside each block = number of top-8 neighbors of
that class. Fused is_ge+accumulate tensor_scalar does each block in one
DVE instruction.
"""

import numpy as np

B, N, D, K, C = 1024, 32768, 64, 8, 6
NCORES = 8
Q = B // NCORES  # queries per core

CHUNK = 512  # matmul moving free dim
MACRO = 2048  # max8 scan chunk
NEG = -1.0e30

_compiled = None


def _plan_layout(y_train: np.ndarray):
    """Class-sort permutation and even-width class blocks, padded to a
    multiple of MACRO columns."""
    perm = np.argsort(y_train, kind="stable")
    counts = np.bincount(y_train, minlength=C)
    widths = [int(c + (c & 1)) for c in counts]  # even block widths
    starts = np.concatenate([[0], np.cumsum(widths)]).astype(int)
    total = int(starts[-1])
    np_cols = ((total + MACRO - 1) // MACRO) * MACRO
    if np_cols < total + 0:
        np_cols += MACRO
    return perm, counts, widths, starts, np_cols


def _build_nc(np_cols: int, block_bounds, finalize: bool = True):
    import concourse.bacc as bacc
    import concourse.mybir as mybir
    from concourse.tile import TileContext

    f32 = mybir.dt.float32
    nc = bacc.Bacc(None, target_bir_lowering=False, debug=False)

    lhsT_d = nc.declare_dram_parameter("lhsT", [D + 1, Q], f32, isOutput=False)
    xm_d = nc.declare_dram_parameter("xm", [D + 1, np_cols], f32, isOutput=False)
    out_d = nc.declare_dram_parameter("out", [Q, C], f32, isOutput=True)

    n_chunks = np_cols // CHUNK
    n_macro = np_cols // MACRO
    per_macro = MACRO // CHUNK

    with TileContext(nc) as tc:
        with (
            tc.tile_pool(name="const", bufs=1) as const_pool,
            tc.tile_pool(name="rhs", bufs=4) as rhs_pool,
            tc.tile_pool(name="psum", bufs=2, space="PSUM") as psum_pool,
            tc.tile_pool(name="keys", bufs=1) as keys_pool,
            tc.tile_pool(name="small", bufs=1) as small_pool,
            tc.tile_pool(name="scr", bufs=2) as scr_pool,
        ):
            lhsT_sb = const_pool.tile([D + 1, Q], f32)
            nc.sync.dma_start(out=lhsT_sb, in_=lhsT_d[:, :])

            keys = keys_pool.tile([Q, np_cols], f32)
            cand = small_pool.tile([Q, n_macro * 8], f32)

            for m in range(n_macro):
                ps = psum_pool.tile([Q, MACRO], f32)
                for j in range(per_macro):
                    c = m * per_macro + j
                    rhs = rhs_pool.tile([D + 1, CHUNK], f32)
                    nc.sync.dma_start(
                        out=rhs, in_=xm_d[:, c * CHUNK : (c + 1) * CHUNK]
                    )
                    nc.tensor.matmul(
                        ps[:, j * CHUNK : (j + 1) * CHUNK],
                        lhsT=lhsT_sb,
                        rhs=rhs,
                        start=True,
                        stop=True,
                    )
                # evacuate PSUM -> SBUF on the scalar engine
                nc.scalar.copy(keys[:, m * MACRO : (m + 1) * MACRO], ps)
                # chunk top-8 candidates
                nc.vector.max(
                    out=cand[:, m * 8 : (m + 1) * 8],
                    in_=keys[:, m * MACRO : (m + 1) * MACRO],
                )

            v8 = small_pool.tile([Q, 8], f32)
            nc.vector.max(out=v8, in_=cand)
            tq = v8[:, 7:8]

            cnt = small_pool.tile([Q, C], f32)
            for ci, (s, e) in enumerate(block_bounds):
                scratch = scr_pool.tile([Q, max(w for _, w in
                                                [(b[0], b[1] - b[0]) for b in block_bounds])],
                                        f32, tag="scratch")
                nc.vector.tensor_scalar(
                    out=scratch[:, : e - s],
                    in0=keys[:, s:e],
                    scalar1=tq,
                    scalar2=None,
                    op0=mybir.AluOpType.is_ge,
                    op1=mybir.AluOpType.add,
                    accum_out=cnt[:, ci : ci + 1],
                )

            tot = small_pool.tile([Q, 1], f32)
            nc.vector.reduce_sum(tot, cnt, axis=mybir.AxisListType.X)
            rec = small_pool.tile([Q, 1], f32)
            nc.vector.reciprocal(rec, tot)
            prob = small_pool.tile([Q, C], f32)
            nc.vector.tensor_scalar(
                out=prob,
                in0=cnt,
                scalar1=rec,
                scalar2=None,
                op0=mybir.AluOpType.mult,
            )
            nc.sync.dma_start(out=out_d[:, :], in_=prob)

    if finalize:
        nc.finalize()
    return nc


def _prepare(x: np.ndarray, X_train: np.ndarray, y_train: np.ndarray):
    perm, counts, widths, starts, np_cols = _plan_layout(y_train)
    Xs = X_train[perm]  # [N, D] class-sorted
    t_sq = np.sum(Xs.astype(np.float32) * Xs.astype(np.float32), axis=1)

    xm = np.full((D + 1, np_cols), 0.0, dtype=np.float32)
    xm[D, :] = NEG  # dummy columns never win
    col = np.zeros(np_cols, dtype=bool)
    # scatter class blocks
    pos = 0
    bounds = []
    for ci in range(C):
        s = int(starts[ci])
        cnt_c = int(counts[ci])
        sel = slice(pos, pos + cnt_c)  # rows of Xs for this class (sorted)
        xm[:D, s : s + cnt_c] = Xs[sel].T
        xm[D, s : s + cnt_c] = -0.5 * t_sq[sel]
        bounds.append((s, s + widths[ci]))
        pos += cnt_c
    return xm, bounds, np_cols


def _in_maps(x: np.ndarray, X_train: np.ndarray, y_train: np.ndarray):
    global _compiled
    xm, bounds, np_cols = _prepare(x, X_train, y_train)
    if _compiled is None:
        _compiled = _build_nc(np_cols, bounds)
    in_maps = []
    for core in range(NCORES):
        xc = x[core * Q : (core + 1) * Q].astype(np.float32)  # [Q, D]
        lhsT = np.concatenate([xc.T, np.ones((1, Q), np.float32)], axis=0)
        in_maps.append({"lhsT": lhsT, "xm": xm})
    return in_maps


def kernel(x: np.ndarray, X_train: np.ndarray, y_train: np.ndarray) -> np.ndarray:
    from concourse.bass_utils import run_bass_kernel_spmd

    in_maps = _in_maps(x, X_train, y_train)
    nc = _compiled

    res = run_bass_kernel_spmd(nc, in_maps, core_ids=list(range(NCORES)))
    out = np.concatenate([res.results[i]["out"] for i in range(NCORES)], axis=0)
    return out.astype(np.float32)



# revision 4
# speedup vs baseline: 1.2273x; 1.2273x over previous
"""KNN classifier layer (B=1024, N=32768, D=64, k=8, C=6) on 8 trn2 cores.

Strategy: shard queries (batch) across the 8 cores, 128 queries per core;
replicate the training set. Per core the ranking key is
  key[q, n] = x_q . X_n - |X_n|^2/2   (monotone decreasing in distance^2)
computed with an fp16 hi/lo split so the PE streams at 1 cycle/row
(fp32 matmul is 4 cycles/row, float32r loses ~2e-3 which flips
near-tie neighbors):
  x = xh + xl, X = Xh + Xl (fp16 parts; products are exact in fp32)
  key ~= xh.Xh + xl.Xh + xh.Xl - (tsqh + tsql)
as 3 accumulating fp16 matmuls per 512-col chunk:
  MM1 [xh;1].[Xh;-tsqh],  MM2 [xl;0].[Xh;-tsqh],  MM3 [xh;1].[Xl;-tsql]
Residual error ~2e-5, well under the minimum 8th/9th-neighbor key gap
(2.4e-4) on gaussian data. X_train is host-sorted by class into 8-col
aligned blocks. Top-8 is one single Max8 pass directly over PSUM per
(class-block x 2048-col PSUM tile) segment; per-class top-8 = Max8 of
that class's segment candidates; t_q = 8th largest over all classes;
counts = is_ge(t_q) sums over each class's top-8. The two fp16 stream
tensors are DMAed on the two HWDGE rings (sync + scalar) in parallel.
"""

import numpy as np

B, N, D, K, C = 1024, 32768, 64, 8, 6
NCORES = 8
Q = B // NCORES  # queries per core

CHUNK = 512    # matmul moving free dim / one PSUM bank (fp32 out)
MACRO = 2048   # PSUM tile width (4 banks) = Max8 scan segment ceiling
STRIPE = 8192  # DMA stripe width (fp16: 16KB/partition, ~1MB/transfer)
NEGF = -60000.0  # finite fp16 filler for padded columns (never wins)

_compiled = None
_compiled_key = None


def _plan_layout(y_train: np.ndarray):
    """Class-sort permutation and 8-col-aligned class blocks; pad the last
    block so the total is a multiple of CHUNK."""
    counts = np.bincount(y_train, minlength=C)
    widths = [max(8, int(-(-c // 8)) * 8) for c in counts]
    total = sum(widths)
    widths[C - 1] += (-total) % CHUNK
    starts = np.concatenate([[0], np.cumsum(widths)]).astype(int)
    np_cols = int(starts[-1])
    # scan segments: intersections of class blocks with the 2048 macro grid
    segs = []  # (macro, class, col_start, width)
    for ci in range(C):
        s, e = int(starts[ci]), int(starts[ci] + widths[ci])
        pos = s
        while pos < e:
            m = pos // MACRO
            w = min((m + 1) * MACRO, e) - pos
            segs.append((m, ci, pos, w))
            pos += w
    segs.sort()
    return counts, widths, starts, np_cols, segs


def _build_nc(np_cols: int, segs):
    import concourse.bacc as bacc
    import concourse.mybir as mybir
    from concourse.tile import TileContext

    f32 = mybir.dt.float32
    f16 = mybir.dt.float16
    nc = bacc.Bacc(None, target_bir_lowering=False, debug=False)

    lhsT_d = nc.declare_dram_parameter("lhsT", [D + 1, 2 * Q], f16, isOutput=False)
    xa_d = nc.declare_dram_parameter("xa", [D + 1, np_cols], f16, isOutput=False)
    xb_d = nc.declare_dram_parameter("xb", [D + 1, np_cols], f16, isOutput=False)
    out_d = nc.declare_dram_parameter("out", [Q, C], f32, isOutput=True)

    nseg = len(segs)
    # candidate slot of each segment in the cand tile, grouped by class
    by_class = [[i for i, sg in enumerate(segs) if sg[1] == c] for c in range(C)]
    slot_of = {}
    off = 0
    class_off = []
    for c in range(C):
        class_off.append(off)
        for i in by_class[c]:
            slot_of[i] = off
            off += 1

    n_macro = -(-np_cols // MACRO)

    with TileContext(nc) as tc:
        with (
            tc.tile_pool(name="const", bufs=1) as const_pool,
            tc.tile_pool(name="sa", bufs=3) as sa_pool,
            tc.tile_pool(name="sb", bufs=3) as sb_pool,
            tc.tile_pool(name="psum", bufs=2, space="PSUM") as psum_pool,
            tc.tile_pool(name="small", bufs=1) as small_pool,
        ):
            w_sb = const_pool.tile([D + 1, 2 * Q], f16)
            nc.sync.dma_start(out=w_sb, in_=lhsT_d[:, :])
            w1 = w_sb[:, 0:Q]
            w2 = w_sb[:, Q : 2 * Q]

            cand = small_pool.tile([Q, nseg * 8], f32)

            stripe_tiles = {}

            def get_stripes(si):
                if si not in stripe_tiles:
                    w = min(STRIPE, np_cols - si * STRIPE)
                    ta = sa_pool.tile([D + 1, w], f16)
                    nc.sync.dma_start(
                        out=ta, in_=xa_d[:, si * STRIPE : si * STRIPE + w]
                    )
                    tb = sb_pool.tile([D + 1, w], f16)
                    nc.scalar.dma_start(
                        out=tb, in_=xb_d[:, si * STRIPE : si * STRIPE + w]
                    )
                    stripe_tiles[si] = (ta, tb)
                return stripe_tiles[si]

            seg_i = 0
            for m in range(n_macro):
                mw = min(MACRO, np_cols - m * MACRO)
                ps = psum_pool.tile([Q, mw], f32)
                for j in range(mw // CHUNK):
                    col = m * MACRO + j * CHUNK
                    si, soff = divmod(col, STRIPE)
                    ta, tb = get_stripes(si)
                    ra = ta[:, soff : soff + CHUNK]
                    rb = tb[:, soff : soff + CHUNK]
                    pc = ps[:, j * CHUNK : (j + 1) * CHUNK]
                    nc.tensor.matmul(pc, lhsT=w1, rhs=ra, start=True, stop=False)
                    nc.tensor.matmul(pc, lhsT=w1, rhs=rb, start=False, stop=False)
                    nc.tensor.matmul(pc, lhsT=w2, rhs=ra, start=False, stop=True)
                # per-(class x macro) segment candidates
                while seg_i < nseg and segs[seg_i][0] == m:
                    _, ci, s, w = segs[seg_i]
                    sl = slot_of[seg_i]
                    nc.vector.max(
                        out=cand[:, sl * 8 : (sl + 1) * 8],
                        in_=ps[:, s - m * MACRO : s - m * MACRO + w],
                    )
                    seg_i += 1

            # per-class top-8 from that class's segment candidates
            all48 = small_pool.tile([Q, C * 8], f32)
            for c in range(C):
                nc.vector.max(
                    out=all48[:, c * 8 : (c + 1) * 8],
                    in_=cand[:, class_off[c] * 8 : (class_off[c] + len(by_class[c])) * 8],
                )

            v8 = small_pool.tile([Q, 8], f32)
            nc.vector.max(out=v8, in_=all48)
            tq = v8[:, 7:8]

            cnt = small_pool.tile([Q, C], f32)
            scr = small_pool.tile([Q, 8], f32, tag="scr")
            for c in range(C):
                nc.vector.tensor_scalar(
                    out=scr,
                    in0=all48[:, c * 8 : (c + 1) * 8],
                    scalar1=tq,
                    scalar2=None,
                    op0=mybir.AluOpType.is_ge,
                    op1=mybir.AluOpType.add,
                    accum_out=cnt[:, c : c + 1],
                )

            tot = small_pool.tile([Q, 1], f32)
            nc.vector.reduce_sum(tot, cnt, axis=mybir.AxisListType.X)
            rec = small_pool.tile([Q, 1], f32)
            nc.vector.reciprocal(rec, tot)
            prob = small_pool.tile([Q, C], f32)
            nc.vector.tensor_scalar(
                out=prob,
                in0=cnt,
                scalar1=rec,
                scalar2=None,
                op0=mybir.AluOpType.mult,
            )
            nc.sync.dma_start(out=out_d[:, :], in_=prob)

    nc.finalize()
    return nc


def _prepare(x: np.ndarray, X_train: np.ndarray, y_train: np.ndarray):
    counts, widths, starts, np_cols, segs = _plan_layout(y_train)
    perm = np.argsort(y_train, kind="stable")
    Xs = X_train[perm].astype(np.float32)  # [N, D] class-sorted
    t = 0.5 * np.sum(Xs.astype(np.float64) * Xs, axis=1).astype(np.float32)

    Xh = Xs.astype(np.float16).astype(np.float32)
    Xl = (Xs - Xh).astype(np.float16)
    th = t.astype(np.float16).astype(np.float32)
    tl = (t - th).astype(np.float16)

    xa = np.full((D + 1, np_cols), 0.0, dtype=np.float16)
    xb = np.full((D + 1, np_cols), 0.0, dtype=np.float16)
    xa[D, :] = NEGF
    xb[D, :] = NEGF
    pos = 0
    for ci in range(C):
        s = int(starts[ci])
        cnt_c = int(counts[ci])
        sel = slice(pos, pos + cnt_c)
        xa[:D, s : s + cnt_c] = Xh[sel].T.astype(np.float16)
        xa[D, s : s + cnt_c] = -th[sel].astype(np.float16)
        xb[:D, s : s + cnt_c] = Xl[sel].T
        xb[D, s : s + cnt_c] = -tl[sel]
        pos += cnt_c
    return xa, xb, np_cols, segs


def _in_maps(x: np.ndarray, X_train: np.ndarray, y_train: np.ndarray):
    global _compiled, _compiled_key
    xa, xb, np_cols, segs = _prepare(x, X_train, y_train)
    key = (np_cols, tuple(segs))
    if _compiled is None or _compiled_key != key:
        _compiled = _build_nc(np_cols, segs)
        _compiled_key = key
    in_maps = []
    xf = x.astype(np.float32)
    xh = xf.astype(np.float16).astype(np.float32)
    xl = (xf - xh).astype(np.float16)
    for core in range(NCORES):
        sel = slice(core * Q, (core + 1) * Q)
        lhsT = np.zeros((D + 1, 2 * Q), dtype=np.float16)
        lhsT[:D, :Q] = xh[sel].T.astype(np.float16)
        lhsT[D, :Q] = 1.0
        lhsT[:D, Q:] = xl[sel].T
        lhsT[D, Q:] = 0.0
        in_maps.append({"lhsT": lhsT, "xa": xa, "xb": xb})
    return in_maps


def kernel(x: np.ndarray, X_train: np.ndarray, y_train: np.ndarray) -> np.ndarray:
    from concourse.bass_utils import run_bass_kernel_spmd

    in_maps = _in_maps(x, X_train, y_train)
    nc = _compiled

    res = run_bass_kernel_spmd(nc, in_maps, core_ids=list(range(NCORES)))
    out = np.concatenate([res.results[i]["out"] for i in range(NCORES)], axis=0)
    return out.astype(np.float32)


# revision 5
# speedup vs baseline: 2.2187x; 1.8077x over previous
"""KNN classifier layer (B=1024, N=32768, D=64, k=8, C=6) on 8 trn2 cores.

Strategy: shard queries (batch) across the 8 cores, 128 queries per core;
replicate the training set. Per core the ranking key is
  key[q, n] = x_q . X_n - |X_n|^2/2   (monotone decreasing in distance^2)
computed exactly-enough with an fp16 hi/lo split (fp16 x fp16 products
are exact in fp32 PSUM; residual ~2e-5 is far under the minimum
8th/9th-neighbor key gap of 2.4e-4):
  key ~= xh.Xh - (tsqh + tsql)  +  (xl.Xh + xh.Xl)
as TWO accumulating fp16 matmuls per 512-col chunk:
  MM_A  K=66  [xh; 1; 1] . [Xh; -tsqh; -tsql]     (start)
  MM_B  K=128 [xl; xh]   . [Xh; Xl]               (stop)
MM_B uses the full 128-row PE array (measured: full-K matmuls warm the
PE clock gate to 2.4 GHz; K<=66 ones stay at 1.2 GHz). X_train is
host-sorted by class into 8-col aligned blocks. Top-8 is one single
Max8 pass directly over PSUM per (class-block x 2048-col PSUM tile)
segment; per-class top-8 = Max8 of that class's segment candidates;
t_q = 8th largest over all classes; counts = is_ge(t_q) sums over each
class's top-8. The two fp16 stream tensors are DMAed on the two HWDGE
rings (sync + scalar) in parallel.
"""

import numpy as np

B, N, D, K, C = 1024, 32768, 64, 8, 6
NCORES = 8
Q = B // NCORES  # queries per core

CHUNK = 512    # matmul moving free dim / one PSUM bank (fp32 out)
MACRO = 2048   # PSUM tile width (4 banks) = Max8 scan segment ceiling
STRIPE = 8192  # DMA stripe width
NEGF = -60000.0  # finite fp16 filler for padded columns (never wins)

_compiled = None
_compiled_key = None


def _plan_layout(y_train: np.ndarray):
    """Class-sort permutation and 8-col-aligned class blocks; pad the last
    block so the total is a multiple of CHUNK."""
    counts = np.bincount(y_train, minlength=C)
    widths = [max(8, int(-(-c // 8)) * 8) for c in counts]
    total = sum(widths)
    widths[C - 1] += (-total) % CHUNK
    starts = np.concatenate([[0], np.cumsum(widths)]).astype(int)
    np_cols = int(starts[-1])
    # scan segments: intersections of class blocks with the 2048 macro grid
    segs = []  # (macro, class, col_start, width)
    for ci in range(C):
        s, e = int(starts[ci]), int(starts[ci] + widths[ci])
        pos = s
        while pos < e:
            m = pos // MACRO
            w = min((m + 1) * MACRO, e) - pos
            segs.append((m, ci, pos, w))
            pos += w
    segs.sort()
    return counts, widths, starts, np_cols, segs


def _build_nc(np_cols: int, segs):
    import concourse.bacc as bacc
    import concourse.mybir as mybir
    from concourse.tile import TileContext

    f32 = mybir.dt.float32
    f16 = mybir.dt.float16
    nc = bacc.Bacc(None, target_bir_lowering=False, debug=False)

    lhsT_d = nc.declare_dram_parameter("lhsT", [D * 2, 2 * Q], f16, isOutput=False)
    xa_d = nc.declare_dram_parameter("xa", [D + 2, np_cols], f16, isOutput=False)
    xb_d = nc.declare_dram_parameter("xb", [2 * D, np_cols], f16, isOutput=False)
    out_d = nc.declare_dram_parameter("out", [Q, C], f32, isOutput=True)

    nseg = len(segs)
    by_class = [[i for i, sg in enumerate(segs) if sg[1] == c] for c in range(C)]
    slot_of = {}
    off = 0
    class_off = []
    for c in range(C):
        class_off.append(off)
        for i in by_class[c]:
            slot_of[i] = off
            off += 1

    n_macro = -(-np_cols // MACRO)

    with TileContext(nc) as tc:
        with (
            tc.tile_pool(name="const", bufs=1) as const_pool,
            tc.tile_pool(name="sa", bufs=3) as sa_pool,
            tc.tile_pool(name="sb", bufs=3) as sb_pool,
            tc.tile_pool(name="psum", bufs=2, space="PSUM") as psum_pool,
            tc.tile_pool(name="small", bufs=1) as small_pool,
        ):
            w_sb = const_pool.tile([D * 2, 2 * Q], f16)
            nc.sync.dma_start(out=w_sb, in_=lhsT_d[:, :])
            wA = w_sb[0 : D + 2, 0:Q]       # [xh; 1; 1]
            wB = w_sb[:, Q : 2 * Q]         # [xl; xh]

            cand = small_pool.tile([Q, nseg * 8], f32)

            stripe_tiles = {}

            def get_stripes(si):
                if si not in stripe_tiles:
                    w = min(STRIPE, np_cols - si * STRIPE)
                    ta = sa_pool.tile([D + 2, w], f16)
                    nc.scalar.dma_start(
                        out=ta, in_=xa_d[:, si * STRIPE : si * STRIPE + w]
                    )
                    tb = sb_pool.tile([2 * D, w], f16)
                    nc.sync.dma_start(
                        out=tb, in_=xb_d[:, si * STRIPE : si * STRIPE + w]
                    )
                    stripe_tiles[si] = (ta, tb)
                return stripe_tiles[si]

            seg_i = 0
            for m in range(n_macro):
                mw = min(MACRO, np_cols - m * MACRO)
                ps = psum_pool.tile([Q, mw], f32)
                for j in range(mw // CHUNK):
                    col = m * MACRO + j * CHUNK
                    si, soff = divmod(col, STRIPE)
                    ta, tb = get_stripes(si)
                    pc = ps[:, j * CHUNK : (j + 1) * CHUNK]
                    nc.tensor.matmul(
                        pc, lhsT=wA, rhs=ta[:, soff : soff + CHUNK],
                        start=True, stop=False,
                    )
                    nc.tensor.matmul(
                        pc, lhsT=wB, rhs=tb[:, soff : soff + CHUNK],
                        start=False, stop=True,
                    )
                while seg_i < nseg and segs[seg_i][0] == m:
                    _, ci, s, w = segs[seg_i]
                    sl = slot_of[seg_i]
                    nc.vector.max(
                        out=cand[:, sl * 8 : (sl + 1) * 8],
                        in_=ps[:, s - m * MACRO : s - m * MACRO + w],
                    )
                    seg_i += 1

            all48 = small_pool.tile([Q, C * 8], f32)
            for c in range(C):
                nc.vector.max(
                    out=all48[:, c * 8 : (c + 1) * 8],
                    in_=cand[:, class_off[c] * 8 : (class_off[c] + len(by_class[c])) * 8],
                )

            v8 = small_pool.tile([Q, 8], f32)
            nc.vector.max(out=v8, in_=all48)
            tq = v8[:, 7:8]

            cnt = small_pool.tile([Q, C], f32)
            scr = small_pool.tile([Q, 8], f32, tag="scr")
            for c in range(C):
                nc.vector.tensor_scalar(
                    out=scr,
                    in0=all48[:, c * 8 : (c + 1) * 8],
                    scalar1=tq,
                    scalar2=None,
                    op0=mybir.AluOpType.is_ge,
                    op1=mybir.AluOpType.add,
                    accum_out=cnt[:, c : c + 1],
                )

            tot = small_pool.tile([Q, 1], f32)
            nc.vector.reduce_sum(tot, cnt, axis=mybir.AxisListType.X)
            rec = small_pool.tile([Q, 1], f32)
            nc.vector.reciprocal(rec, tot)
            prob = small_pool.tile([Q, C], f32)
            nc.vector.tensor_scalar(
                out=prob,
                in0=cnt,
                scalar1=rec,
                scalar2=None,
                op0=mybir.AluOpType.mult,
            )
            nc.sync.dma_start(out=out_d[:, :], in_=prob)

    nc.finalize()
    return nc


def _prepare(x: np.ndarray, X_train: np.ndarray, y_train: np.ndarray):
    counts, widths, starts, np_cols, segs = _plan_layout(y_train)
    perm = np.argsort(y_train, kind="stable")
    Xs = X_train[perm].astype(np.float32)  # [N, D] class-sorted
    t = 0.5 * np.sum(Xs.astype(np.float64) * Xs, axis=1).astype(np.float32)

    Xh = Xs.astype(np.float16).astype(np.float32)
    Xl = (Xs - Xh).astype(np.float16)
    th = t.astype(np.float16).astype(np.float32)
    tl = (t - th).astype(np.float16)

    # xa = [Xh; -tsqh; -tsql]  (66 rows)
    xa = np.zeros((D + 2, np_cols), dtype=np.float16)
    xa[D, :] = NEGF
    xa[D + 1, :] = NEGF
    # xb = [Xh; Xl]  (128 rows)
    xb = np.zeros((2 * D, np_cols), dtype=np.float16)
    pos = 0
    for ci in range(C):
        s = int(starts[ci])
        cnt_c = int(counts[ci])
        sel = slice(pos, pos + cnt_c)
        xa[:D, s : s + cnt_c] = Xh[sel].T.astype(np.float16)
        xa[D, s : s + cnt_c] = -th[sel].astype(np.float16)
        xa[D + 1, s : s + cnt_c] = -tl[sel]
        xb[:D, s : s + cnt_c] = Xh[sel].T.astype(np.float16)
        xb[D:, s : s + cnt_c] = Xl[sel].T
        pos += cnt_c
    return xa, xb, np_cols, segs


def _in_maps(x: np.ndarray, X_train: np.ndarray, y_train: np.ndarray):
    global _compiled, _compiled_key
    xa, xb, np_cols, segs = _prepare(x, X_train, y_train)
    key = (np_cols, tuple(segs))
    if _compiled is None or _compiled_key != key:
        _compiled = _build_nc(np_cols, segs)
        _compiled_key = key
    in_maps = []
    xf = x.astype(np.float32)
    xh = xf.astype(np.float16).astype(np.float32)
    xl = (xf - xh).astype(np.float16)
    for core in range(NCORES):
        sel = slice(core * Q, (core + 1) * Q)
        lhsT = np.zeros((2 * D, 2 * Q), dtype=np.float16)
        # wA = [xh; 1; 1] in rows 0..D+1, cols 0..Q
        lhsT[:D, :Q] = xh[sel].T.astype(np.float16)
        lhsT[D, :Q] = 1.0
        lhsT[D + 1, :Q] = 1.0
        # wB = [xl; xh] in rows 0..2D, cols Q..2Q
        lhsT[:D, Q:] = xl[sel].T
        lhsT[D:, Q:] = xh[sel].T.astype(np.float16)
        in_maps.append({"lhsT": lhsT, "xa": xa, "xb": xb})
    return in_maps


def kernel(x: np.ndarray, X_train: np.ndarray, y_train: np.ndarray) -> np.ndarray:
    from concourse.bass_utils import run_bass_kernel_spmd

    in_maps = _in_maps(x, X_train, y_train)
    nc = _compiled

    res = run_bass_kernel_spmd(nc, in_maps, core_ids=list(range(NCORES)))
    out = np.concatenate([res.results[i]["out"] for i in range(NCORES)], axis=0)
    return out.astype(np.float32)


# revision 6
# speedup vs baseline: 2.2302x; 1.0052x over previous
"""KNN classifier layer (B=1024, N=32768, D=64, k=8, C=6) on 8 trn2 cores.

Strategy: shard queries (batch) across the 8 cores, 128 queries per core;
replicate the training set. Per core the ranking key is
  key[q, n] = x_q . X_n - |X_n|^2/2   (monotone decreasing in distance^2)
computed exactly-enough with an fp16 hi/lo split (fp16 x fp16 products
are exact in fp32 PSUM; residual ~2e-5 is far under the minimum
8th/9th-neighbor key gap of 2.4e-4):
  key ~= xh.Xh - (tsqh + tsql)  +  (xl.Xh + xh.Xl)
as TWO accumulating fp16 matmuls per 512-col chunk:
  MM_A  K=66  [xh; 1; 1] . [Xh; -tsqh; -tsql]     (start)
  MM_B  K=128 [xl; xh]   . [Xh; Xl]               (stop)
MM_B uses the full 128-row PE array (measured: full-K matmuls warm the
PE clock gate to 2.4 GHz; K<=66 ones stay at 1.2 GHz). X_train is
host-sorted by class into 8-col aligned blocks. Top-8 is one single
Max8 pass directly over PSUM per (class-block x 2048-col PSUM tile)
segment; per-class top-8 = Max8 of that class's segment candidates;
t_q = 8th largest over all classes; counts = is_ge(t_q) sums over each
class's top-8. The two fp16 stream tensors are DMAed on the two HWDGE
rings (sync + scalar) in parallel.
"""

import numpy as np

B, N, D, K, C = 1024, 32768, 64, 8, 6
NCORES = 8
Q = B // NCORES  # queries per core

CHUNK = 512    # matmul moving free dim / one PSUM bank (fp32 out)
MACRO = 2048   # PSUM tile width (4 banks) = Max8 scan segment ceiling
STRIPE = 2048  # DMA stripe width
NEGF = -60000.0  # finite fp16 filler for padded columns (never wins)

_compiled = None
_compiled_key = None


def _plan_layout(y_train: np.ndarray):
    """Class-sort permutation and 8-col-aligned class blocks; pad the last
    block so the total is a multiple of CHUNK."""
    counts = np.bincount(y_train, minlength=C)
    widths = [max(8, int(-(-c // 8)) * 8) for c in counts]
    total = sum(widths)
    widths[C - 1] += (-total) % CHUNK
    starts = np.concatenate([[0], np.cumsum(widths)]).astype(int)
    np_cols = int(starts[-1])
    # scan segments: intersections of class blocks with the 2048 macro grid
    segs = []  # (macro, class, col_start, width)
    for ci in range(C):
        s, e = int(starts[ci]), int(starts[ci] + widths[ci])
        pos = s
        while pos < e:
            m = pos // MACRO
            w = min((m + 1) * MACRO, e) - pos
            segs.append((m, ci, pos, w))
            pos += w
    segs.sort()
    return counts, widths, starts, np_cols, segs


def _build_nc(np_cols: int, segs):
    import concourse.bacc as bacc
    import concourse.mybir as mybir
    from concourse.tile import TileContext

    f32 = mybir.dt.float32
    f16 = mybir.dt.float16
    nc = bacc.Bacc(None, target_bir_lowering=False, debug=False)

    lhsT_d = nc.declare_dram_parameter("lhsT", [D * 2, 2 * Q], f16, isOutput=False)
    xa_d = nc.declare_dram_parameter("xa", [D + 2, np_cols], f16, isOutput=False)
    xb_d = nc.declare_dram_parameter("xb", [2 * D, np_cols], f16, isOutput=False)
    out_d = nc.declare_dram_parameter("out", [Q, C], f32, isOutput=True)

    nseg = len(segs)
    by_class = [[i for i, sg in enumerate(segs) if sg[1] == c] for c in range(C)]
    slot_of = {}
    off = 0
    class_off = []
    for c in range(C):
        class_off.append(off)
        for i in by_class[c]:
            slot_of[i] = off
            off += 1

    n_macro = -(-np_cols // MACRO)

    with TileContext(nc) as tc:
        with (
            tc.tile_pool(name="const", bufs=1) as const_pool,
            tc.tile_pool(name="sa", bufs=5) as sa_pool,
            tc.tile_pool(name="sb", bufs=5) as sb_pool,
            tc.tile_pool(name="psum", bufs=2, space="PSUM") as psum_pool,
            tc.tile_pool(name="small", bufs=1) as small_pool,
        ):
            w_sb = const_pool.tile([D * 2, 2 * Q], f16)
            nc.sync.dma_start(out=w_sb, in_=lhsT_d[:, :])
            wA = w_sb[0 : D + 2, 0:Q]       # [xh; 1; 1]
            wB = w_sb[:, Q : 2 * Q]         # [xl; xh]

            cand = small_pool.tile([Q, nseg * 8], f32)

            stripe_tiles = {}

            def get_stripes(si):
                if si not in stripe_tiles:
                    w = min(STRIPE, np_cols - si * STRIPE)
                    eng = nc.sync if si % 2 == 0 else nc.scalar
                    ta = sa_pool.tile([D + 2, w], f16)
                    eng.dma_start(
                        out=ta, in_=xa_d[:, si * STRIPE : si * STRIPE + w]
                    )
                    tb = sb_pool.tile([2 * D, w], f16)
                    eng.dma_start(
                        out=tb, in_=xb_d[:, si * STRIPE : si * STRIPE + w]
                    )
                    stripe_tiles[si] = (ta, tb)
                return stripe_tiles[si]

            seg_i = 0
            for m in range(n_macro):
                mw = min(MACRO, np_cols - m * MACRO)
                ps = psum_pool.tile([Q, mw], f32)
                for j in range(mw // CHUNK):
                    col = m * MACRO + j * CHUNK
                    si, soff = divmod(col, STRIPE)
                    ta, tb = get_stripes(si)
                    pc = ps[:, j * CHUNK : (j + 1) * CHUNK]
                    nc.tensor.matmul(
                        pc, lhsT=wA, rhs=ta[:, soff : soff + CHUNK],
                        start=True, stop=False,
                    )
                    nc.tensor.matmul(
                        pc, lhsT=wB, rhs=tb[:, soff : soff + CHUNK],
                        start=False, stop=True,
                    )
                while seg_i < nseg and segs[seg_i][0] == m:
                    _, ci, s, w = segs[seg_i]
                    sl = slot_of[seg_i]
                    nc.vector.max(
                        out=cand[:, sl * 8 : (sl + 1) * 8],
                        in_=ps[:, s - m * MACRO : s - m * MACRO + w],
                    )
                    seg_i += 1

            all48 = small_pool.tile([Q, C * 8], f32)
            for c in range(C):
                nc.vector.max(
                    out=all48[:, c * 8 : (c + 1) * 8],
                    in_=cand[:, class_off[c] * 8 : (class_off[c] + len(by_class[c])) * 8],
                )

            v8 = small_pool.tile([Q, 8], f32)
            nc.vector.max(out=v8, in_=all48)
            tq = v8[:, 7:8]

            cnt = small_pool.tile([Q, C], f32)
            scr = small_pool.tile([Q, 8], f32, tag="scr")
            for c in range(C):
                nc.vector.tensor_scalar(
                    out=scr,
                    in0=all48[:, c * 8 : (c + 1) * 8],
                    scalar1=tq,
                    scalar2=None,
                    op0=mybir.AluOpType.is_ge,
                    op1=mybir.AluOpType.add,
                    accum_out=cnt[:, c : c + 1],
                )

            tot = small_pool.tile([Q, 1], f32)
            nc.vector.reduce_sum(tot, cnt, axis=mybir.AxisListType.X)
            rec = small_pool.tile([Q, 1], f32)
            nc.vector.reciprocal(rec, tot)
            prob = small_pool.tile([Q, C], f32)
            nc.vector.tensor_scalar(
                out=prob,
                in0=cnt,
                scalar1=rec,
                scalar2=None,
                op0=mybir.AluOpType.mult,
            )
            nc.sync.dma_start(out=out_d[:, :], in_=prob)

    nc.finalize()
    return nc


def _prepare(x: np.ndarray, X_train: np.ndarray, y_train: np.ndarray):
    counts, widths, starts, np_cols, segs = _plan_layout(y_train)
    perm = np.argsort(y_train, kind="stable")
    Xs = X_train[perm].astype(np.float32)  # [N, D] class-sorted
    t = 0.5 * np.sum(Xs.astype(np.float64) * Xs, axis=1).astype(np.float32)

    Xh = Xs.astype(np.float16).astype(np.float32)
    Xl = (Xs - Xh).astype(np.float16)
    th = t.astype(np.float16).astype(np.float32)
    tl = (t - th).astype(np.float16)

    # xa = [Xh; -tsqh; -tsql]  (66 rows)
    xa = np.zeros((D + 2, np_cols), dtype=np.float16)
    xa[D, :] = NEGF
    xa[D + 1, :] = NEGF
    # xb = [Xh; Xl]  (128 rows)
    xb = np.zeros((2 * D, np_cols), dtype=np.float16)
    pos = 0
    for ci in range(C):
        s = int(starts[ci])
        cnt_c = int(counts[ci])
        sel = slice(pos, pos + cnt_c)
        xa[:D, s : s + cnt_c] = Xh[sel].T.astype(np.float16)
        xa[D, s : s + cnt_c] = -th[sel].astype(np.float16)
        xa[D + 1, s : s + cnt_c] = -tl[sel]
        xb[:D, s : s + cnt_c] = Xh[sel].T.astype(np.float16)
        xb[D:, s : s + cnt_c] = Xl[sel].T
        pos += cnt_c
    return xa, xb, np_cols, segs


def _in_maps(x: np.ndarray, X_train: np.ndarray, y_train: np.ndarray):
    global _compiled, _compiled_key
    xa, xb, np_cols, segs = _prepare(x, X_train, y_train)
    key = (np_cols, tuple(segs))
    if _compiled is None or _compiled_key != key:
        _compiled = _build_nc(np_cols, segs)
        _compiled_key = key
    in_maps = []
    xf = x.astype(np.float32)
    xh = xf.astype(np.float16).astype(np.float32)
    xl = (xf - xh).astype(np.float16)
    for core in range(NCORES):
        sel = slice(core * Q, (core + 1) * Q)
        lhsT = np.zeros((2 * D, 2 * Q), dtype=np.float16)
        # wA = [xh; 1; 1] in rows 0..D+1, cols 0..Q
        lhsT[:D, :Q] = xh[sel].T.astype(np.float16)
        lhsT[D, :Q] = 1.0
        lhsT[D + 1, :Q] = 1.0
        # wB = [xl; xh] in rows 0..2D, cols Q..2Q
        lhsT[:D, Q:] = xl[sel].T
        lhsT[D:, Q:] = xh[sel].T.astype(np.float16)
        in_maps.append({"lhsT": lhsT, "xa": xa, "xb": xb})
    return in_maps


def kernel(x: np.ndarray, X_train: np.ndarray, y_train: np.ndarray) -> np.ndarray:
    from concourse.bass_utils import run_bass_kernel_spmd

    in_maps = _in_maps(x, X_train, y_train)
    nc = _compiled

    res = run_bass_kernel_spmd(nc, in_maps, core_ids=list(range(NCORES)))
    out = np.concatenate([res.results[i]["out"] for i in range(NCORES)], axis=0)
    return out.astype(np.float32)


# revision 10
# speedup vs baseline: 2.4135x; 1.0822x over previous
"""KNN classifier layer (B=1024, N=32768, D=64, k=8, C=6) on 8 trn2 cores.

Strategy: shard queries (batch) across the 8 cores, 128 queries per core;
replicate the training set. Per core the ranking key is
  key[q, n] = x_q . X_n - |X_n|^2/2   (monotone decreasing in distance^2)
computed exactly-enough with an fp16 hi/lo split (fp16 x fp16 products
are exact in fp32 PSUM; residual ~2e-5 is far under the minimum
8th/9th-neighbor key gap of 2.4e-4):
  key ~= xh.Xh - (tsqh + tsql)  +  (xl.Xh + xh.Xl)
as TWO accumulating fp16 matmuls per 512-col chunk:
  MM_A  K=66  [xh; 1; 1] . [Xh; -tsqh; -tsql]     (start)
  MM_B  K=128 [xl; xh]   . [Xh; Xl]               (stop)
MM_B uses the full 128-row PE array (measured: full-K matmuls warm the
PE clock gate to 2.4 GHz; K<=66 ones stay at 1.2 GHz). X_train is
host-sorted by class into 8-col aligned blocks. Top-8 is one single
Max8 pass directly over PSUM per (class-block x 2048-col PSUM tile)
segment; per-class top-8 = Max8 of that class's segment candidates;
t_q = 8th largest over all classes; counts = is_ge(t_q) sums over each
class's top-8. The two fp16 stream tensors are DMAed on the two HWDGE
rings (sync + scalar) in parallel.
"""

import numpy as np

B, N, D, K, C = 1024, 32768, 64, 8, 6
NCORES = 8
Q = B // NCORES  # queries per core

CHUNK = 512    # matmul moving free dim / one PSUM bank (fp32 out)
MACRO = 2048   # PSUM tile width (4 banks) = Max8 scan segment ceiling
STRIPE = 2048  # DMA stripe width
NEGF = -60000.0  # finite fp16 filler for padded columns (never wins)

_compiled = None
_compiled_key = None


def _plan_layout(y_train: np.ndarray):
    """Class-sort permutation and 8-col-aligned class blocks; pad the last
    block so the total is a multiple of CHUNK."""
    counts = np.bincount(y_train, minlength=C)
    widths = [max(8, int(-(-c // 8)) * 8) for c in counts]
    total = sum(widths)
    widths[C - 1] += (-total) % CHUNK
    starts = np.concatenate([[0], np.cumsum(widths)]).astype(int)
    np_cols = int(starts[-1])
    # scan segments: intersections of class blocks with the 2048 macro grid
    segs = []  # (macro, class, col_start, width)
    for ci in range(C):
        s, e = int(starts[ci]), int(starts[ci] + widths[ci])
        pos = s
        while pos < e:
            m = pos // MACRO
            w = min((m + 1) * MACRO, e) - pos
            segs.append((m, ci, pos, w))
            pos += w
    segs.sort()
    return counts, widths, starts, np_cols, segs


def _build_nc(np_cols: int, segs):
    import concourse.bacc as bacc
    import concourse.mybir as mybir
    from concourse.tile import TileContext

    f32 = mybir.dt.float32
    f16 = mybir.dt.float16
    nc = bacc.Bacc(None, target_bir_lowering=False, debug=False)

    lhsT_d = nc.declare_dram_parameter("lhsT", [D * 2, 2 * Q], f16, isOutput=False)
    xa_d = nc.declare_dram_parameter("xa", [D + 2, np_cols], f16, isOutput=False)
    xb_d = nc.declare_dram_parameter("xb", [2 * D, np_cols], f16, isOutput=False)
    out_d = nc.declare_dram_parameter("out", [Q, C], f32, isOutput=True)

    nseg = len(segs)
    by_class = [[i for i, sg in enumerate(segs) if sg[1] == c] for c in range(C)]
    slot_of = {}
    off = 0
    class_off = []
    for c in range(C):
        class_off.append(off)
        for i in by_class[c]:
            slot_of[i] = off
            off += 1

    n_macro = -(-np_cols // MACRO)

    with TileContext(nc) as tc:
        with (
            tc.tile_pool(name="const", bufs=1) as const_pool,
            tc.tile_pool(name="sa", bufs=5) as sa_pool,
            tc.tile_pool(name="sb", bufs=5) as sb_pool,
            tc.tile_pool(name="psum", bufs=2, space="PSUM") as psum_pool,
            tc.tile_pool(name="small", bufs=1) as small_pool,
        ):
            w_sb = const_pool.tile([D * 2, 2 * Q], f16)
            nc.sync.dma_start(out=w_sb, in_=lhsT_d[:, :])
            wA = w_sb[0 : D + 2, 0:Q]       # [xh; 1; 1]
            wB = w_sb[:, Q : 2 * Q]         # [xl; xh]

            cand = small_pool.tile([Q, nseg * 8], f32)

            stripe_tiles = {}

            def get_stripes(si):
                if si not in stripe_tiles:
                    w = min(STRIPE, np_cols - si * STRIPE)
                    eng_a = nc.scalar if si % 2 == 0 else nc.sync
                    eng_b = nc.sync if si % 2 == 0 else nc.scalar
                    ta = sa_pool.tile([D + 2, w], f16)
                    eng_a.dma_start(
                        out=ta, in_=xa_d[:, si * STRIPE : si * STRIPE + w]
                    )
                    tb = sb_pool.tile([2 * D, w], f16)
                    eng_b.dma_start(
                        out=tb, in_=xb_d[:, si * STRIPE : si * STRIPE + w]
                    )
                    stripe_tiles[si] = (ta, tb)
                return stripe_tiles[si]

            all48 = small_pool.tile([Q, C * 8], f32)
            # per-class reduction fires as soon as the class's last segment
            # has been scanned (classes are contiguous column blocks)
            last_seg_of_class = {c: max(by_class[c]) for c in range(C)}

            seg_i = 0
            for m in range(n_macro):
                mw = min(MACRO, np_cols - m * MACRO)
                ps = psum_pool.tile([Q, mw], f32)
                for j in range(mw // CHUNK):
                    col = m * MACRO + j * CHUNK
                    si, soff = divmod(col, STRIPE)
                    ta, tb = get_stripes(si)
                    pc = ps[:, j * CHUNK : (j + 1) * CHUNK]
                    nc.tensor.matmul(
                        pc, lhsT=wA, rhs=ta[:, soff : soff + CHUNK],
                        start=True, stop=False,
                    )
                    nc.tensor.matmul(
                        pc, lhsT=wB, rhs=tb[:, soff : soff + CHUNK],
                        start=False, stop=True,
                    )
                while seg_i < nseg and segs[seg_i][0] == m:
                    _, ci, s, w = segs[seg_i]
                    sl = slot_of[seg_i]
                    nc.vector.max(
                        out=cand[:, sl * 8 : (sl + 1) * 8],
                        in_=ps[:, s - m * MACRO : s - m * MACRO + w],
                    )
                    if seg_i == last_seg_of_class[ci]:
                        nc.vector.max(
                            out=all48[:, ci * 8 : (ci + 1) * 8],
                            in_=cand[
                                :,
                                class_off[ci] * 8 : (class_off[ci] + len(by_class[ci])) * 8,
                            ],
                        )
                    seg_i += 1

            v8 = small_pool.tile([Q, 8], f32)
            nc.vector.max(out=v8, in_=all48)
            tq = v8[:, 7:8]

            bits = small_pool.tile([Q, C, 8], f32)
            nc.vector.tensor_scalar(
                out=bits,
                in0=all48,
                scalar1=tq,
                scalar2=None,
                op0=mybir.AluOpType.is_ge,
            )
            cnt = small_pool.tile([Q, C], f32)
            nc.vector.reduce_sum(cnt, bits, axis=mybir.AxisListType.X)

            tot = small_pool.tile([Q, 1], f32)
            nc.vector.reduce_sum(tot, cnt, axis=mybir.AxisListType.X)
            rec = small_pool.tile([Q, 1], f32)
            nc.vector.reciprocal(rec, tot)
            prob = small_pool.tile([Q, C], f32)
            nc.vector.tensor_scalar(
                out=prob,
                in0=cnt,
                scalar1=rec,
                scalar2=None,
                op0=mybir.AluOpType.mult,
            )
            nc.sync.dma_start(out=out_d[:, :], in_=prob)

    nc.finalize()
    return nc


def _prepare(x: np.ndarray, X_train: np.ndarray, y_train: np.ndarray):
    counts, widths, starts, np_cols, segs = _plan_layout(y_train)
    perm = np.argsort(y_train, kind="stable")
    Xs = X_train[perm].astype(np.float32)  # [N, D] class-sorted
    t = 0.5 * np.sum(Xs.astype(np.float64) * Xs, axis=1).astype(np.float32)

    Xh = Xs.astype(np.float16).astype(np.float32)
    Xl = (Xs - Xh).astype(np.float16)
    th = t.astype(np.float16).astype(np.float32)
    tl = (t - th).astype(np.float16)

    # xa = [Xh; -tsqh; -tsql]  (66 rows)
    xa = np.zeros((D + 2, np_cols), dtype=np.float16)
    xa[D, :] = NEGF
    xa[D + 1, :] = NEGF
    # xb = [Xh; Xl]  (128 rows)
    xb = np.zeros((2 * D, np_cols), dtype=np.float16)
    pos = 0
    for ci in range(C):
        s = int(starts[ci])
        cnt_c = int(counts[ci])
        sel = slice(pos, pos + cnt_c)
        xa[:D, s : s + cnt_c] = Xh[sel].T.astype(np.float16)
        xa[D, s : s + cnt_c] = -th[sel].astype(np.float16)
        xa[D + 1, s : s + cnt_c] = -tl[sel]
        xb[:D, s : s + cnt_c] = Xh[sel].T.astype(np.float16)
        xb[D:, s : s + cnt_c] = Xl[sel].T
        pos += cnt_c
    return xa, xb, np_cols, segs


def _in_maps(x: np.ndarray, X_train: np.ndarray, y_train: np.ndarray):
    global _compiled, _compiled_key
    xa, xb, np_cols, segs = _prepare(x, X_train, y_train)
    key = (np_cols, tuple(segs))
    if _compiled is None or _compiled_key != key:
        _compiled = _build_nc(np_cols, segs)
        _compiled_key = key
    in_maps = []
    xf = x.astype(np.float32)
    xh = xf.astype(np.float16).astype(np.float32)
    xl = (xf - xh).astype(np.float16)
    for core in range(NCORES):
        sel = slice(core * Q, (core + 1) * Q)
        lhsT = np.zeros((2 * D, 2 * Q), dtype=np.float16)
        # wA = [xh; 1; 1] in rows 0..D+1, cols 0..Q
        lhsT[:D, :Q] = xh[sel].T.astype(np.float16)
        lhsT[D, :Q] = 1.0
        lhsT[D + 1, :Q] = 1.0
        # wB = [xl; xh] in rows 0..2D, cols Q..2Q
        lhsT[:D, Q:] = xl[sel].T
        lhsT[D:, Q:] = xh[sel].T.astype(np.float16)
        in_maps.append({"lhsT": lhsT, "xa": xa, "xb": xb})
    return in_maps


def kernel(x: np.ndarray, X_train: np.ndarray, y_train: np.ndarray) -> np.ndarray:
    from concourse.bass_utils import run_bass_kernel_spmd

    in_maps = _in_maps(x, X_train, y_train)
    nc = _compiled

    res = run_bass_kernel_spmd(nc, in_maps, core_ids=list(range(NCORES)))
    out = np.concatenate([res.results[i]["out"] for i in range(NCORES)], axis=0)
    return out.astype(np.float32)
